# revision 29
# baseline (speedup 1.0000x reference)
# Trainium2 Bass kernel for nn_NetSparse1 (topk_masking).
#
# Computes: log_softmax( relu(x @ (w1*m1).T) @ (w2*m2).T ) where m1/m2 are
# top-50%-|score| masks (GetSubnetEP semantics, stable-sort tie handling).
#
# Strategy (data-parallel over 8 NeuronCores, batch dim sharded):
#   host: compute the exact reference masks, pre-mask the weights, and
#         quantize x / w1m to TRN fp8 e4m3 (max 240; ml_dtypes.float8_e4m3
#         is bit-identical for |v| <= 240) with power-of-two scales
#         sx=32, sw=512; the 2^-14 product scale is folded into the relu.
#         w2m stays bf16 (layer 2 is tiny).
#   device (per core, 2048 batch rows):
#     layer 1 runs fp8 DoubleRow matmuls (K packed 256 per MM, ~1.5x bf16
#     throughput): for each batch block [512] and hidden chunk [128],
#     3 DR matmuls cover k<768; the k-remainder (16 rows) is replicated at
#     partition bases 0/32/64/96 so 4 hidden chunks' remainder MMs run
#     concurrently in distinct PE row groups. relu+scale on DVE -> bf16 h.
#     layer 2 packs 4 hidden chunks into one PE pass via 4x column tiling
#     (M=10 each, col groups 0..3 of one PSUM bank), deferred one group so
#     the PE never waits on the relu. Per batch block the 4 col-group
#     partials are summed on DVE and the log-softmax tail (PE transpose,
#     exp/ln on ACT, no max-shift needed: logits are O(5)) is interleaved
#     into the next block's matmul stream so only the last tail is exposed.
# No collectives needed; host concatenates the 8 per-core outputs.

import numpy as np
import ml_dtypes

import concourse.bass as bass
import concourse.tile as tile
from concourse import bacc, mybir
from concourse.bass_utils import run_bass_kernel_spmd

N_CORES = 8
B = 16384
BC = B // N_CORES      # 2048 batch rows per core
IN_DIM = 784
HIDDEN = 8192
OUT_DIM = 10
SPARSITY = 0.5

P = 128
KT = 6                 # full 128-row k-tiles (768)
K_LAST = IN_DIM - KT * P  # 16
NDR = KT // 2          # 3 DoubleRow matmuls per output chunk
HC = HIDDEN // P       # 64 hidden chunks
BB = 512               # batch block (PSUM free dim)
NBB = BC // BB         # 4
NG = HC // 4           # 16 groups of 4 hidden chunks
NMB = 8                # w1 DMA column blocks (1024 wide, 8 hc each)
MBW = HIDDEN // NMB    # 1024

SX = 32.0              # x fp8 scale
SW = 512.0             # w1 fp8 scale
INV_S = 1.0 / (SX * SW)  # 2^-14, folded into relu

F32 = mybir.dt.float32
BF16 = mybir.dt.bfloat16
F8 = mybir.dt.float8e4

_BF16 = ml_dtypes.bfloat16
_E4M3 = ml_dtypes.float8_e4m3

N_WARM = 16


def _build_nc():
    nc = bacc.Bacc("TRN2")

    GW = MBW // 2  # 512 hidden cols per w1 DMA piece == one hc group
    xs_d = nc.dram_tensor("xs_d", (P, KT, BC), F8, kind="ExternalInput")
    xrem_d = nc.dram_tensor("xrem_d", (K_LAST, BC), F8, kind="ExternalInput")
    w1_d = nc.dram_tensor("w1_d", (P, NG, KT, GW), F8, kind="ExternalInput")
    w1rem_d = nc.dram_tensor("w1rem_d", (K_LAST, HIDDEN), F8,
                             kind="ExternalInput")
    w2_d = nc.dram_tensor("w2_d", (P, HC, OUT_DIM), BF16,
                          kind="ExternalInput")
    sel_d = nc.dram_tensor("sel_d", (P, OUT_DIM), BF16, kind="ExternalInput")
    # block layout [bb, p, i, o] -> host restores [2048, 10]
    out = nc.dram_tensor("out", (NBB, P, NBB, OUT_DIM), F32,
                         kind="ExternalOutput")

    DR = mybir.MatmulPerfMode.DoubleRow

    with tile.TileContext(nc) as tc:
        with (
            tc.tile_pool(name="singles", bufs=1) as singles,
            tc.tile_pool(name="hts", bufs=8) as hts,
            tc.tile_pool(name="tails", bufs=2) as tails,
            tc.tile_pool(name="psh", bufs=6, space=bass.MemorySpace.PSUM) as psh,
            tc.tile_pool(name="psl", bufs=1, space=bass.MemorySpace.PSUM) as psl,
            tc.tile_pool(name="ptp", bufs=1, space=bass.MemorySpace.PSUM) as ptp,
        ):
            # resident SBUF tensors; w1s layout mirrors w1_d so every DMA
            # piece is contiguous per partition (no tiny-packet storms)
            xs = singles.tile([P, KT, BC], F8, tag="xs")
            xrem = singles.tile([P, BC], F8, tag="xrem")
            w1s = singles.tile([P, NG, KT, GW], F8, tag="w1s")
            w1rem = singles.tile([P, HIDDEN], F8, tag="w1rem")
            w2s = singles.tile([P, HC, OUT_DIM], BF16, tag="w2s")
            sel = singles.tile([P, OUT_DIM], BF16, tag="sel")

            # DMA schedule: earliest-needed first; the vector engine serves
            # as a 4th queue for the two pieces that gate the first group.
            # The k-remainder rows are uploaded once and replicated to
            # partition bases 32/64/96 by SBUF->SBUF DMA (doubling).
            def w1_piece(eng, g):
                eng.dma_start(w1s[:, g], w1_d[:, g])

            nc.sync.dma_start(xs[:, 0:1, :], xs_d[:, 0:1, :])
            nc.gpsimd.dma_start(xs[:, 1:3, :], xs_d[:, 1:3, :])
            nc.scalar.dma_start(xs[:, 3:6, :], xs_d[:, 3:6, :])
            w1_piece(nc.sync, 0)
            nc.gpsimd.dma_start(xrem[0:K_LAST], xrem_d[:])
            nc.gpsimd.dma_start(xrem[32 : 32 + K_LAST], xrem[0:K_LAST])
            nc.gpsimd.dma_start(xrem[64:128], xrem[0:64])
            nc.gpsimd.dma_start(w1rem[0:K_LAST], w1rem_d[:])
            nc.gpsimd.dma_start(w1rem[32 : 32 + K_LAST], w1rem[0:K_LAST])
            nc.gpsimd.dma_start(w1rem[64:128], w1rem[0:64])
            w1_piece(nc.sync, 1)
            nc.scalar.dma_start(w2s, w2_d[:])
            w1_piece(nc.gpsimd, 2)
            w1_piece(nc.scalar, 3)
            for g in range(4, NG):
                eng = (nc.sync, nc.gpsimd, nc.scalar)[(g - 4) % 3]
                w1_piece(eng, g)
            nc.scalar.dma_start(sel, sel_d[:])

            # zero bias for activations
            zb = singles.tile([P, 1], F32, tag="zb")
            nc.vector.memset(zb, 0.0)

            # PE warmup: dependency-free bf16 matmul chain so the HAM
            # clock-gate is at K=8/8 and the initial DMAs are covered
            wz = singles.tile([P, BB], BF16, tag="wz")
            nc.vector.memset(wz, 0.0)
            warm = psh.tile([P, BB], F32, tag="ph", name="warm")
            for i in range(N_WARM):
                nc.tensor.matmul(warm, wz[:, :P], wz, start=(i == 0),
                                 stop=(i == N_WARM - 1))

            # ---- main loop -------------------------------------------------
            # bb-outer so each batch block's logits finish 1/4 through and
            # its softmax tail overlaps the next block's matmuls.
            lg = None
            prev = []     # deferred L2 (col-tiled) for the previous group
            tailq = []    # deferred tail pieces of the previous batch block

            def flush_l2(g_items):
                for ht, hc, cur_lg, g in reversed(g_items):
                    j = hc % 4
                    base = 32 * j
                    nc.tensor.matmul(
                        cur_lg[base : base + OUT_DIM, :],
                        w2s[:, hc, :],
                        ht,
                        start=(g == 0),
                        stop=(g == NG - 1),
                        tile_position=(0, base),
                    )

            def make_tail(bb, cur_lg, last=False):
                # returns a list of closures, each a tail piece to interleave.
                # For the last block the 4 selection matmuls get their own
                # psum banks (ph pool is free by then) so the MM/exp chain
                # pipelines instead of serializing on one bank.
                st = {}

                def pt_tile(i):
                    if last:
                        return psh.tile([P, NBB * OUT_DIM], F32, tag="ph",
                                        name=f"pt_{bb}_{i}")
                    if i == 0:
                        st["pt1"] = ptp.tile([P, NBB * OUT_DIM], F32,
                                             tag="pt", name=f"pt_{bb}")
                    return st["pt1"]

                def piece_sum():
                    lgsb = tails.tile([P, BB], BF16, tag="lgsb",
                                      name=f"lgsb_{bb}")
                    nc.vector.tensor_copy(lgsb, cur_lg)
                    st["lgsb"] = lgsb
                    st["e"] = tails.tile([P, OUT_DIM], F32, tag="e",
                                         name=f"e_{bb}")
                    st["s"] = tails.tile([P, NBB], F32, tag="s",
                                         name=f"s_{bb}")
                    st["pt"] = {}

                def piece_t(i):
                    def run():
                        pt = pt_tile(i)
                        st["pt"][i] = pt
                        osl = slice(i * OUT_DIM, (i + 1) * OUT_DIM)
                        nc.tensor.matmul(pt[:, osl],
                                         st["lgsb"][:, i * P : (i + 1) * P],
                                         sel, start=True, stop=True)
                        nc.scalar.activation(
                            out=st["e"], in_=pt[:, osl],
                            func=mybir.ActivationFunctionType.Exp,
                            bias=zb, accum_out=st["s"][:, i : i + 1])
                    return run

                def piece_out():
                    ls = tails.tile([P, NBB], F32, tag="ls", name=f"ls_{bb}")
                    nc.scalar.activation(out=ls, in_=st["s"],
                                         func=mybir.ActivationFunctionType.Ln,
                                         bias=zb)
                    ot = tails.tile([P, NBB, OUT_DIM], F32, tag="ot",
                                    name=f"ot_{bb}")
                    for i in range(NBB):
                        nc.vector.tensor_scalar(
                            out=ot[:, i, :],
                            in0=st["pt"][i][:, i * OUT_DIM : (i + 1) * OUT_DIM],
                            scalar1=ls[:, i : i + 1], scalar2=None,
                            op0=mybir.AluOpType.subtract)
                    nc.gpsimd.dma_start(out[bb], ot)

                return [piece_sum, piece_t(0), piece_t(1), piece_t(2),
                        piece_t(3), piece_out]

            for bb in range(NBB):
                bsl = slice(bb * BB, (bb + 1) * BB)
                lg = psl.tile([P, BB], F32, tag="lg", name=f"lg_{bb}")
                for g in range(NG):
                    phs = [psh.tile([P, BB], F32, tag="ph",
                                    name=f"ph_{bb}_{g}_{j}") for j in range(4)]
                    for j in range(4):
                        for i in range(NDR):
                            nc.tensor.matmul(
                                phs[j],
                                w1s[:, g, 2 * i : 2 * i + 2,
                                    j * P : (j + 1) * P],
                                xs[:, 2 * i : 2 * i + 2, bsl],
                                start=(i == 0),
                                stop=False,
                                perf_mode=DR,
                            )
                    # 4 k-remainder MMs run concurrently in PE row groups
                    for j in range(4):
                        hc = 4 * g + j
                        base = 32 * j
                        nc.tensor.matmul(
                            phs[j],
                            w1rem[base : base + K_LAST, hc * P : (hc + 1) * P],
                            xrem[base : base + K_LAST, bsl],
                            start=False,
                            stop=True,
                            tile_position=(base, 0) if base == 96 else None,
                        )
                    # relu (+2^-14 scale) -> bf16, alternating DVE / ACT so
                    # neither engine becomes the straggler
                    cur = []
                    for j in range(4):
                        ht = hts.tile([P, BB], BF16, tag="ht")
                        if j % 2 == 0:
                            nc.vector.tensor_scalar(
                                out=ht, in0=phs[j], scalar1=INV_S,
                                scalar2=0.0,
                                op0=mybir.AluOpType.mult,
                                op1=mybir.AluOpType.max)
                        else:
                            nc.scalar.activation(
                                out=ht, in_=phs[j],
                                func=mybir.ActivationFunctionType.Relu,
                                bias=zb, scale=INV_S)
                        cur.append((ht, 4 * g + j, lg, g))
                    flush_l2(prev)
                    prev = cur
                    # interleave one tail piece of the previous batch block
                    if tailq:
                        tailq.pop(0)()
                    if g == 0:
                        # zero the logits bank so partitions the col-tiled
                        # matmuls never touch read back as 0 (not stale NaN)
                        # in the selection matmul
                        nc.vector.memset(lg, 0.0)
                flush_l2(prev)
                prev = []
                tailq.extend(make_tail(bb, lg, last=(bb == NBB - 1)))
            # last block's tail runs at the end
            for piece in tailq:
                piece()

    nc.compile()
    return nc


_NC = None


def _get_nc():
    global _NC
    if _NC is None:
        _NC = _build_nc()
    return _NC


def _exact_mask(scores):
    """GetSubnetEP mask: top 50% of |scores| under stable (value, index)
    order, matching jnp.argsort's stable tie handling exactly."""
    s32 = np.asarray(scores, dtype=np.float32)
    a = np.abs(s32).ravel()
    n = a.size
    j = int((1.0 - SPARSITY) * n)
    t = np.partition(a, j)[j]
    lt = int((a < t).sum())
    ties = np.flatnonzero(a == t)  # ascending flat index == stable order
    mask = a > t
    mask[ties[j - lt :]] = True
    assert int(mask.sum()) == n - j
    return mask.reshape(s32.shape)


def _q8(a, scale):
    return np.clip(a * np.float32(scale), -224.0, 224.0).astype(_E4M3)


def _prepare_inputs(x, w1, scores1, w2, scores2):
    x = np.asarray(x, dtype=np.float32)
    w1 = np.asarray(w1, dtype=np.float32)
    w2 = np.asarray(w2, dtype=np.float32)

    w1m = w1 * _exact_mask(scores1)        # [8192, 784]
    w2m = w2 * _exact_mask(scores2)        # [10, 8192]

    w1q = _q8(w1m.T, SW)                   # [784, 8192] fp8
    # w1_d[p, g, kt, ci] = w1q[kt*128 + p, g*512 + ci]
    w1_dr = np.ascontiguousarray(
        w1q[: KT * P]
        .reshape(KT, P, NG, MBW // 2)
        .transpose(1, 2, 0, 3))
    w1rem_dr = np.ascontiguousarray(w1q[KT * P :])  # [16, 8192]

    # w2_d[p, c, o] = w2m[o, c*128 + p]
    w2_dr = np.ascontiguousarray(
        w2m.T.reshape(HC, P, OUT_DIM).transpose(1, 0, 2)).astype(_BF16)

    sel_dr = np.zeros((P, OUT_DIM), _BF16)
    for j in range(4):
        for o in range(OUT_DIM):
            sel_dr[32 * j + o, o] = 1.0

    xq = _q8(x.T, SX)                      # [784, 16384] fp8
    common = {"w1_d": w1_dr, "w1rem_d": w1rem_dr, "w2_d": w2_dr,
              "sel_d": sel_dr}
    in_maps = []
    for c in range(N_CORES):
        xc = xq[:, c * BC : (c + 1) * BC]  # [784, 2048]
        xs_dr = np.ascontiguousarray(
            xc[: KT * P].reshape(KT, P, BC).transpose(1, 0, 2))
        xrem_dr = np.ascontiguousarray(xc[KT * P :])  # [16, 2048]
        m = dict(common)
        m["xs_d"] = xs_dr
        m["xrem_d"] = xrem_dr
        in_maps.append(m)
    return in_maps


def run(inputs, trace=False, **kwargs):
    """Run the kernel; returns (output ndarray, BassKernelResults)."""
    nc = _get_nc()
    in_maps = _prepare_inputs(**inputs)
    res = run_bass_kernel_spmd(nc, in_maps, core_ids=list(range(N_CORES)),
                               trace=trace, **kwargs)
    # out block layout [bb, p, i, o] -> row b = bb*512 + i*128 + p
    outp = np.concatenate(
        [r["out"].transpose(0, 2, 1, 3).reshape(BC, OUT_DIM)
         for r in res.results], axis=0)
    return np.ascontiguousarray(outp.astype(np.float32)), res


def kernel(x, w1, scores1, w2, scores2):
    outp, _ = run(dict(x=x, w1=w1, scores1=scores1, w2=w2, scores2=scores2))
    return outp
